# revision 8
# baseline (speedup 1.0000x reference)
"""Bayesian-STDP classic scan kernel for Trainium2 (8 NeuronCores).

Math (per time-batch i, reference semantics):
    corr_i = spk_i^T @ psp_i                      # (O, I)
    mu_i   = 1 / nk_i          (per-O, nk_i = 1 + sum_{j<i} tos_j)
    w     += mu_i*C*corr_i*exp(-w) - mu_i*tos_i
    b     += exp(-b)*(mub_i*toss_i*tos_i) - mub_i*toss_i
    nk    += tos_i

Only the w -> exp(-w) -> w chain is sequential; corr and every scalar are
input-only.  Restructure used here:

  * O-sharding: core c owns output rows [128c, 128c+128) -> W is a single
    (128, 2048) partition tile; all per-O quantities are per-partition
    scalars.
  * spk is pre-scaled by (C*mu_i) on host, so the PE matmul directly
    produces mu_i*C*corr_i.
  * The "- mu_i*tos_i" rank-1 subtraction is folded away with the
    substitution wt_i = w_i + Cw_i  (Cw_i = sum_{j<i} mu_j*tos_j, per-O):
        wt_{i+1} = wt_i + (mu_i*C*corr_i) * exp(-wt_i + Cw_i)
    exp(-wt + Cw) is one ScalarE activation with per-partition bias.
  * wt lives in PSUM (4 banks); the "+=" is a float32r identity matmul
    accumulating into the bank (PE), so VectorE does exactly one
    tensor_tensor multiply (corr * e) per 512-column chunk.
  * b gets the same treatment with scalar cumulative offsets; its update is
    one fused scalar_tensor_tensor op.
"""

import os
import sys
from contextlib import ExitStack

import numpy as np

for _p in ("/opt/trn_rl_repo", "/opt/pypackages"):
    if os.path.isdir(_p) and _p not in sys.path:
        sys.path.insert(0, _p)

import concourse.bacc as bacc
import concourse.bass as bass
import concourse.mybir as mybir
import concourse.tile as tile
from concourse.bass_utils import run_bass_kernel_spmd
from concourse.bass import ds

F32 = mybir.dt.float32
F32R = mybir.dt.float32r
EXP = mybir.ActivationFunctionType.Exp
MULT = mybir.AluOpType.mult
ADD = mybir.AluOpType.add
SUB = mybir.AluOpType.subtract

# Problem constants (hardcoded; kernel.py must be self-contained).
TIME_BATCH = 10
BASE_MU = 1.0
BASE_MU_BIAS = 1.0
C_CONST = 1.0
T_FULL, I_FULL, O_FULL = 10240, 2048, 1024
N_CORES = 8
OS = O_FULL // N_CORES  # 128 output rows per core == one partition tile


class Cfg:
    def __init__(self, iters=1024, tb=TIME_BATCH, ifull=I_FULL, ch=512, b=16):
        self.iters = iters      # number of scan iterations
        self.tb = tb            # time-batch length (matmul contraction K)
        self.ifull = ifull      # input-feature columns held per core
        self.ch = ch            # column chunk (one PSUM bank of fp32)
        self.b = b              # iterations per For_i trip / scalar block
        self.nch = ifull // ch
        self.nblk = iters // b
        assert ifull % ch == 0 and iters % b == 0
        assert self.nch <= 4, "wt + corr must fit in 8 PSUM banks"


def build_nc(cfg: Cfg, debug=False, repeats=1):
    """repeats>1 re-runs the whole scan (incl. state re-init) that many
    times — identical I/O, used only for differential wall-clock timing."""
    nc = bacc.Bacc(
        "TRN2",
        target_bir_lowering=False,
        debug=debug,
        enable_asserts=True,
        num_devices=N_CORES,
    )
    tb, ifull, ch, nch, b, nblk = cfg.tb, cfg.ifull, cfg.ch, cfg.nch, cfg.b, cfg.nblk

    psp_d = nc.dram_tensor("psp", [nblk, b, tb, ifull], F32R, kind="ExternalInput")
    spks_d = nc.dram_tensor("spks", [nblk, b, tb, OS], F32R, kind="ExternalInput")
    w0_d = nc.dram_tensor("w0", [OS, ifull], F32, kind="ExternalInput")
    b0_d = nc.dram_tensor("b0", [OS, 1], F32, kind="ExternalInput")
    cw_d = nc.dram_tensor("cw", [nblk, OS, b], F32, kind="ExternalInput")
    cb_d = nc.dram_tensor("cb", [nblk, OS, b], F32, kind="ExternalInput")
    sb1_d = nc.dram_tensor("sb1", [nblk, OS, b], F32, kind="ExternalInput")
    cwf_d = nc.dram_tensor("cwf", [OS, 1], F32, kind="ExternalInput")
    cbf_d = nc.dram_tensor("cbf", [OS, 1], F32, kind="ExternalInput")
    id_d = nc.dram_tensor("ident", [OS, OS], F32R, kind="ExternalInput")
    id32_d = nc.dram_tensor("ident32", [OS, OS], F32, kind="ExternalInput")
    wout_d = nc.dram_tensor("w_out", [OS, ifull], F32, kind="ExternalOutput")
    bout_d = nc.dram_tensor("b_out", [OS, 1], F32, kind="ExternalOutput")

    with tile.TileContext(nc) as tc, ExitStack() as ctx:
        const_pool = ctx.enter_context(tc.tile_pool(name="const", bufs=1))
        out_pool = ctx.enter_context(tc.tile_pool(name="outp", bufs=1))
        psum_w = ctx.enter_context(
            tc.tile_pool(name="psw", bufs=1, space=bass.MemorySpace.PSUM)
        )
        psum_corr = ctx.enter_context(
            tc.tile_pool(name="psc", bufs=4, space=bass.MemorySpace.PSUM)
        )
        psp_pool = ctx.enter_context(tc.tile_pool(name="pspp", bufs=4))
        spk_pool = ctx.enter_context(tc.tile_pool(name="spkp", bufs=4))
        sc_pool = ctx.enter_context(tc.tile_pool(name="scp", bufs=2))
        e_pool = ctx.enter_context(tc.tile_pool(name="ep", bufs=8))
        t1_pool = ctx.enter_context(tc.tile_pool(name="t1p", bufs=8))

        ident = const_pool.tile([OS, OS], F32R)
        nc.sync.dma_start(ident[:], id_d[:])
        ident32 = const_pool.tile([OS, OS], F32)
        nc.sync.dma_start(ident32[:], id32_d[:])
        w0s = const_pool.tile([OS, ifull], F32)
        nc.sync.dma_start(w0s[:], w0_d[:])
        btil = const_pool.tile([OS, 1], F32)
        nc.sync.dma_start(btil[:], b0_d[:])
        cwf_t = const_pool.tile([OS, 1], F32)
        nc.sync.dma_start(cwf_t[:], cwf_d[:])
        cbf_t = const_pool.tile([OS, 1], F32)
        nc.sync.dma_start(cbf_t[:], cbf_d[:])

        wps = psum_w.tile([OS, ifull], F32)

        def emit_scan():
            # wt lives in PSUM for the whole scan; init by exact fp32
            # identity matmul (sets has_written so start=False accumulates).
            if repeats > 1:
                nc.sync.dma_start(btil[:], b0_d[:])
            for c in range(nch):
                nc.tensor.matmul(
                    wps[:, bass.ts(c, ch)],
                    ident32[:],
                    w0s[:, bass.ts(c, ch)],
                    start=True,
                    stop=True,
                )
            emit_loop()

        def emit_loop():
          with tc.For_i(0, nblk, 1) as blk:
            cwt = sc_pool.tile([OS, b], F32)
            nc.sync.dma_start(cwt[:], cw_d[ds(blk, 1)][0])
            cbt = sc_pool.tile([OS, b], F32)
            nc.sync.dma_start(cbt[:], cb_d[ds(blk, 1)][0])
            sb1t = sc_pool.tile([OS, b], F32)
            nc.sync.dma_start(sb1t[:], sb1_d[ds(blk, 1)][0])
            psp_blk = psp_d[ds(blk, 1)]
            spk_blk = spks_d[ds(blk, 1)]

            for k in range(b):
                pspt = psp_pool.tile([tb, ifull], F32R)
                nc.sync.dma_start(pspt[:], psp_blk[0, k])
                spkt = spk_pool.tile([tb, OS], F32R)
                nc.sync.dma_start(spkt[:], spk_blk[0, k])

                for c in range(nch):
                    cols = bass.ts(c, ch)
                    corr = psum_corr.tile([OS, ch], F32)
                    nc.tensor.matmul(
                        corr[:],
                        spkt[:],
                        pspt[:, cols],
                        start=True,
                        stop=True,
                    )
                    e = e_pool.tile([OS, ch], F32)
                    nc.scalar.activation(
                        e[:], wps[:, cols], EXP, bias=cwt[:, k : k + 1], scale=-1.0
                    )
                    t1 = t1_pool.tile([OS, ch], F32R)
                    nc.vector.tensor_mul(t1[:], corr[:], e[:])
                    nc.tensor.matmul(
                        wps[:, cols],
                        ident[:],
                        t1[:],
                        start=False,
                        stop=True,
                        skip_group_check=True,
                    )

                eb = e_pool.tile([OS, 1], F32)
                nc.scalar.activation(
                    eb[:], btil[:], EXP, bias=cbt[:, k : k + 1], scale=-1.0
                )
                nc.vector.scalar_tensor_tensor(
                    btil[:], eb[:], sb1t[:, k : k + 1], btil[:], op0=MULT, op1=ADD
                )

        if repeats == 1:
            emit_scan()
        else:
            with tc.For_i(0, repeats, 1):
                emit_scan()

        # un-substitute: w = wt - Cw_final, b = bt - Cb_final
        wsb = out_pool.tile([OS, ifull], F32)
        nc.vector.tensor_scalar(wsb[:], wps[:], cwf_t[:, 0:1], None, op0=SUB)
        nc.sync.dma_start(wout_d[:], wsb[:])
        bsb = out_pool.tile([OS, 1], F32)
        nc.vector.tensor_scalar(bsb[:], btil[:], cbf_t[:, 0:1], None, op0=SUB)
        nc.sync.dma_start(bout_d[:], bsb[:])

    nc.compile()
    return nc


def host_prepare(input_psp, output_spikes, weights, biases, N_k, cfg: Cfg):
    """Compute scan-independent arrays (fp64) and slice per-core in_maps."""
    iters, tb, ifull, b, nblk = cfg.iters, cfg.tb, cfg.ifull, cfg.b, cfg.nblk
    o_full = weights.shape[0]
    n_cores = o_full // OS

    psp_b = np.asarray(input_psp, np.float32).reshape(iters, tb, ifull)
    spk_b = np.asarray(output_spikes, np.float32).reshape(iters, tb, o_full)

    tos = spk_b.sum(axis=1, dtype=np.float64)          # (iters, O)
    toss = tos.sum(axis=1)                             # (iters,)
    csum = np.cumsum(tos, axis=0)
    nk0 = np.asarray(N_k, np.float64).reshape(1, o_full)
    nk = np.concatenate([nk0, nk0 + csum[:-1]], axis=0)   # nk before iter i
    muw = BASE_MU / nk                                 # (iters, O)

    spks = (spk_b.astype(np.float64) * (C_CONST * muw)[:, None, :]).astype(np.float32)

    cw_terms = muw * tos
    cw_all = np.concatenate(
        [np.zeros((1, o_full)), np.cumsum(cw_terms, axis=0)], axis=0
    )  # (iters+1, O): Cw_i
    snk = nk.sum(axis=1)                               # sum_o nk_i[o]
    mub = BASE_MU_BIAS / snk                           # (iters,)
    sb1 = (mub * toss)[:, None] * tos                  # (iters, O)
    sb2 = mub * toss
    cb_all = np.concatenate([[0.0], np.cumsum(sb2)])   # (iters+1,)

    ident = np.eye(OS, dtype=np.float32)
    w_f32 = np.asarray(weights, np.float32)
    b_f32 = np.asarray(biases, np.float32)

    # psp blocks are shared by every core
    psp_arr = np.ascontiguousarray(psp_b.reshape(nblk, b, tb, ifull))
    cb_arr = np.ascontiguousarray(
        np.broadcast_to(
            cb_all[:iters].reshape(nblk, 1, b).astype(np.float32), (nblk, OS, b)
        )
    )
    cbf_arr = np.full((OS, 1), cb_all[iters], np.float32)

    in_maps = []
    for c in range(n_cores):
        oc = slice(c * OS, (c + 1) * OS)
        in_maps.append(
            {
                "psp": psp_arr,
                "spks": np.ascontiguousarray(
                    spks[:, :, oc].reshape(nblk, b, tb, OS)
                ),
                "w0": np.ascontiguousarray(w_f32[oc]),
                "b0": np.ascontiguousarray(b_f32[oc].reshape(OS, 1)),
                "cw": np.ascontiguousarray(
                    cw_all[:iters, oc].reshape(nblk, b, OS).transpose(0, 2, 1)
                ).astype(np.float32),
                "cb": cb_arr,
                "sb1": np.ascontiguousarray(
                    sb1[:, oc].reshape(nblk, b, OS).transpose(0, 2, 1)
                ).astype(np.float32),
                "cwf": cw_all[iters, oc].reshape(OS, 1).astype(np.float32),
                "cbf": cbf_arr,
                "ident": ident,
                "ident32": ident,
            }
        )
    return in_maps


_NC_CACHE = {}


def _get_nc(cfg: Cfg, repeats=1):
    key = (cfg.iters, cfg.tb, cfg.ifull, cfg.ch, cfg.b, repeats)
    if key not in _NC_CACHE:
        _NC_CACHE[key] = build_nc(cfg, repeats=repeats)
    return _NC_CACHE[key]


def run_on_cores(in_maps, cfg: Cfg, trace=False, repeats=1, **kw):
    nc = _get_nc(cfg, repeats=repeats)
    return run_bass_kernel_spmd(
        nc, in_maps, core_ids=list(range(len(in_maps))), trace=trace, **kw
    )


def kernel(input_psp, output_spikes, weights, biases, N_k):
    cfg = Cfg()
    in_maps = host_prepare(input_psp, output_spikes, weights, biases, N_k, cfg)
    res = run_on_cores(in_maps, cfg)
    w = np.concatenate([res.results[c]["w_out"] for c in range(N_CORES)], axis=0)
    b = np.concatenate(
        [res.results[c]["b_out"][:, 0] for c in range(N_CORES)], axis=0
    )
    return (w, b)


# revision 21
# speedup vs baseline: 27.2075x; 27.2075x over previous
"""Bayesian-STDP classic scan kernel for Trainium2 (8 NeuronCores).

Math (per time-batch i, reference semantics):
    corr_i = spk_i^T @ psp_i                      # (O, I)
    mu_i   = 1 / nk_i          (per-O, nk_i = 1 + sum_{j<i} tos_j)
    w     += mu_i*C*corr_i*exp(-w) - mu_i*tos_i
    b     += exp(-b)*(mub_i*toss_i*tos_i) - mub_i*toss_i
    nk    += tos_i

Only the w -> exp(-w) -> w chain is sequential; corr and every scalar are
input-only.  Restructure used here:

  * O-sharding: core c owns output rows [128c, 128c+128) -> W is a single
    (128, 2048) partition tile; all per-O quantities are per-partition
    scalars.
  * spk is pre-scaled by (C*mu_i) on host, so the PE matmul directly
    produces mu_i*C*corr_i.
  * The "- mu_i*tos_i" rank-1 subtraction is folded away with the
    substitution wt_i = w_i + Cw_i  (Cw_i = sum_{j<i} mu_j*tos_j, per-O):
        wt_{i+1} = wt_i + (mu_i*C*corr_i) * exp(-wt_i + Cw_i)
    exp(-wt + Cw) is one ScalarE activation with per-partition bias.
  * wt lives in PSUM (4 banks); the "+=" is a float32r identity matmul
    accumulating into the bank (PE), so VectorE does exactly one
    tensor_tensor multiply (corr * e) per 512-column chunk.
  * b gets the same treatment with scalar cumulative offsets; its update is
    one fused scalar_tensor_tensor op.
"""

import os
import sys
from contextlib import ExitStack

import numpy as np

for _p in ("/opt/trn_rl_repo", "/opt/pypackages"):
    if os.path.isdir(_p) and _p not in sys.path:
        sys.path.insert(0, _p)

import concourse.bacc as bacc
import concourse.bass as bass
import concourse.mybir as mybir
import concourse.tile as tile
from concourse.bass_utils import run_bass_kernel_spmd
from concourse.bass import ds
from concourse.tile import add_dep_helper

F32 = mybir.dt.float32
F32R = mybir.dt.float32r
EXP = mybir.ActivationFunctionType.Exp
MULT = mybir.AluOpType.mult
ADD = mybir.AluOpType.add
SUB = mybir.AluOpType.subtract

# Problem constants (hardcoded; kernel.py must be self-contained).
TIME_BATCH = 10
BASE_MU = 1.0
BASE_MU_BIAS = 1.0
C_CONST = 1.0
T_FULL, I_FULL, O_FULL = 10240, 2048, 1024
N_CORES = 8
OS = O_FULL // N_CORES  # 128 output rows per core == one partition tile


class Cfg:
    def __init__(
        self,
        iters=1024,
        tb=TIME_BATCH,
        ifull=I_FULL,
        ch=512,
        b=32,
        staggered=False,
        hints=False,
    ):
        self.iters = iters      # number of scan iterations
        self.tb = tb            # time-batch length (matmul contraction K)
        self.ifull = ifull      # input-feature columns held per core
        self.ch = ch            # column chunk (one PSUM bank of fp32)
        self.b = b              # iterations per For_i trip / scalar block
        self.staggered = staggered  # staggered_reset on the For_i back-edge
        self.hints = hints          # branch-prefetch hint engines
        self.nch = ifull // ch
        self.nblk = iters // b
        assert ifull % ch == 0 and iters % b == 0
        assert self.nch <= 4, "wt + corr must fit in 8 PSUM banks"

    def key(self):
        return (self.iters, self.tb, self.ifull, self.ch, self.b,
                self.staggered, self.hints)


def build_nc(cfg: Cfg, debug=False, repeats=1):
    """repeats>1 re-runs the whole scan (incl. state re-init) that many
    times — identical I/O, used only for differential wall-clock timing."""
    nc = bacc.Bacc(
        "TRN2",
        target_bir_lowering=False,
        debug=debug,
        enable_asserts=True,
        num_devices=N_CORES,
    )
    tb, ifull, ch, nch, b, nblk = cfg.tb, cfg.ifull, cfg.ch, cfg.nch, cfg.b, cfg.nblk

    # one extra zero block at the end: the software-pipelined corr prefetch
    # reads one iteration ahead
    psp_d = nc.dram_tensor("psp", [nblk + 1, b, tb, ifull], F32R, kind="ExternalInput")
    spks_d = nc.dram_tensor("spks", [nblk + 1, b, tb, OS], F32R, kind="ExternalInput")
    w0_d = nc.dram_tensor("w0", [OS, ifull], F32, kind="ExternalInput")
    b0_d = nc.dram_tensor("b0", [OS, 1], F32, kind="ExternalInput")
    cw_d = nc.dram_tensor("cw", [nblk, OS, b], F32, kind="ExternalInput")
    cb_d = nc.dram_tensor("cb", [nblk, OS, b], F32, kind="ExternalInput")
    sb1_d = nc.dram_tensor("sb1", [nblk, OS, b], F32, kind="ExternalInput")
    cwf_d = nc.dram_tensor("cwf", [OS, 1], F32, kind="ExternalInput")
    cbf_d = nc.dram_tensor("cbf", [OS, 1], F32, kind="ExternalInput")
    id_d = nc.dram_tensor("ident", [OS, OS], F32R, kind="ExternalInput")
    id32_d = nc.dram_tensor("ident32", [OS, OS], F32, kind="ExternalInput")
    wout_d = nc.dram_tensor("w_out", [OS, ifull], F32, kind="ExternalOutput")
    bout_d = nc.dram_tensor("b_out", [OS, 1], F32, kind="ExternalOutput")

    with tile.TileContext(nc) as tc, ExitStack() as ctx:
        const_pool = ctx.enter_context(tc.tile_pool(name="const", bufs=1))
        out_pool = ctx.enter_context(tc.tile_pool(name="outp", bufs=1))
        psum_w = ctx.enter_context(
            tc.tile_pool(name="psw", bufs=1, space=bass.MemorySpace.PSUM)
        )
        psum_corr = ctx.enter_context(
            tc.tile_pool(name="psc", bufs=1, space=bass.MemorySpace.PSUM)
        )
        psp_pool = ctx.enter_context(tc.tile_pool(name="pspp", bufs=4))
        spk_pool = ctx.enter_context(tc.tile_pool(name="spkp", bufs=4))
        sc_pool = ctx.enter_context(tc.tile_pool(name="scp", bufs=2))
        e_pool = ctx.enter_context(tc.tile_pool(name="ep", bufs=8))
        t1_pool = ctx.enter_context(tc.tile_pool(name="t1p", bufs=8))

        ident = const_pool.tile([OS, OS], F32R)
        nc.sync.dma_start(ident[:], id_d[:])
        ident32 = const_pool.tile([OS, OS], F32)
        nc.sync.dma_start(ident32[:], id32_d[:])
        w0s = const_pool.tile([OS, ifull], F32)
        nc.sync.dma_start(w0s[:], w0_d[:])
        btil = const_pool.tile([OS, 1], F32)
        nc.sync.dma_start(btil[:], b0_d[:])
        cwf_t = const_pool.tile([OS, 1], F32)
        nc.sync.dma_start(cwf_t[:], cwf_d[:])
        cbf_t = const_pool.tile([OS, 1], F32)
        nc.sync.dma_start(cbf_t[:], cbf_d[:])

        wps = [
            psum_w.tile([OS, ch], F32, name=f"wps{c}", tag=f"wps{c}")
            for c in range(nch)
        ]

        def emit_scan():
            # wt lives in PSUM for the whole scan; init by exact fp32
            # identity matmul (sets has_written so start=False accumulates).
            if repeats > 1:
                nc.sync.dma_start(btil[:], b0_d[:])
            for c in range(nch):
                nc.tensor.matmul(
                    wps[c][:],
                    ident32[:],
                    w0s[:, bass.ts(c, ch)],
                    start=True,
                    stop=True,
                )
            emit_loop()

        corr_t = [
            psum_corr.tile([OS, ch], F32, name=f"corrbuf{c}", tag=f"corrbuf{c}")
            for c in range(nch)
        ]

        def emit_loop():
            # prologue: corr for iteration 0; in-loop, each body computes
            # iteration k+1's corr right after k's mul frees the buffer
            pspt0 = psp_pool.tile([tb, ifull], F32R, name="pspt", tag="pspt")
            nc.sync.dma_start(pspt0[:], psp_d[0, 0])
            spkt0 = spk_pool.tile([tb, OS], F32R, name="spkt", tag="spkt")
            nc.sync.dma_start(spkt0[:], spks_d[0, 0])
            for c in range(nch):
                nc.tensor.matmul(
                    corr_t[c][:], spkt0[:], pspt0[:, bass.ts(c, ch)],
                    start=True, stop=True,
                )

            loop_kw = {}
            if cfg.staggered:
                loop_kw["staggered_reset"] = True
            if cfg.hints:
                loop_kw["hint_engines"] = (
                    mybir.EngineType.PE,
                    mybir.EngineType.Activation,
                    mybir.EngineType.DVE,
                    mybir.EngineType.SP,
                )
            with tc.For_i(0, nblk, 1, **loop_kw) as blk:
                cwt = sc_pool.tile([OS, b], F32)
                nc.sync.dma_start(cwt[:], cw_d[ds(blk, 1)][0])
                cbt = sc_pool.tile([OS, b], F32)
                nc.sync.dma_start(cbt[:], cb_d[ds(blk, 1)][0])
                sb1t = sc_pool.tile([OS, b], F32)
                nc.sync.dma_start(sb1t[:], sb1_d[ds(blk, 1)][0])
                psp_blk = psp_d[ds(blk, 1)]
                spk_blk = spks_d[ds(blk, 1)]
                psp_nblk = psp_d[ds(blk + 1, 1)]
                spk_nblk = spks_d[ds(blk + 1, 1)]

                prev_corr = None
                for k in range(b):
                    # fetch iteration k+1's inputs; compute its corr this
                    # body so PE never head-of-line blocks next iter's muls
                    psrc = psp_blk[0, k + 1] if k < b - 1 else psp_nblk[0, 0]
                    ssrc = spk_blk[0, k + 1] if k < b - 1 else spk_nblk[0, 0]
                    pspt = psp_pool.tile([tb, ifull], F32R, name="pspt", tag="pspt")
                    nc.sync.dma_start(pspt[:], psrc)
                    spkt = spk_pool.tile([tb, OS], F32R, name="spkt", tag="spkt")
                    nc.sync.dma_start(spkt[:], ssrc)

                    for c in range(nch):
                        e = e_pool.tile([OS, ch], F32, name="e", tag="e")
                        nc.scalar.activation(
                            e[:], wps[c][:], EXP, bias=cwt[:, k : k + 1], scale=-1.0
                        )
                        t1 = t1_pool.tile([OS, ch], F32R, name="t1", tag="t1")
                        nc.vector.tensor_mul(t1[:], corr_t[c][:], e[:])
                        acc_i = nc.tensor.matmul(
                            wps[c][:],
                            ident[:],
                            t1[:],
                            start=False,
                            stop=True,
                            skip_group_check=True,
                        )
                        corr_i = nc.tensor.matmul(
                            corr_t[c][:],
                            spkt[:],
                            pspt[:, bass.ts(c, ch)],
                            start=True,
                            stop=True,
                        )
                        # force PE alternation acc(c) -> corr(c) -> acc(c+1)
                        add_dep_helper(
                            corr_i.ins, acc_i.ins, sync=False, reason="pe-pair"
                        )
                        if prev_corr is not None:
                            add_dep_helper(
                                acc_i.ins, prev_corr.ins, sync=False,
                                reason="pe-chain",
                            )
                        prev_corr = corr_i

                    eb = e_pool.tile([OS, 1], F32, name="eb", tag="eb")
                    nc.scalar.activation(
                        eb[:], btil[:], EXP, bias=cbt[:, k : k + 1], scale=-1.0
                    )
                    nc.vector.scalar_tensor_tensor(
                        btil[:], eb[:], sb1t[:, k : k + 1], btil[:], op0=MULT, op1=ADD
                    )

        if repeats == 1:
            emit_scan()
        else:
            with tc.For_i(0, repeats, 1):
                emit_scan()

        # un-substitute: w = wt - Cw_final, b = bt - Cb_final
        wsb = out_pool.tile([OS, ifull], F32)
        for c in range(nch):
            nc.vector.tensor_scalar(
                wsb[:, bass.ts(c, ch)], wps[c][:], cwf_t[:, 0:1], None, op0=SUB
            )
        nc.sync.dma_start(wout_d[:], wsb[:])
        bsb = out_pool.tile([OS, 1], F32)
        nc.vector.tensor_scalar(bsb[:], btil[:], cbf_t[:, 0:1], None, op0=SUB)
        nc.sync.dma_start(bout_d[:], bsb[:])

    nc.compile()
    return nc


def host_prepare(input_psp, output_spikes, weights, biases, N_k, cfg: Cfg):
    """Compute scan-independent arrays (fp64) and slice per-core in_maps."""
    iters, tb, ifull, b, nblk = cfg.iters, cfg.tb, cfg.ifull, cfg.b, cfg.nblk
    o_full = weights.shape[0]
    n_cores = o_full // OS

    psp_b = np.asarray(input_psp, np.float32).reshape(iters, tb, ifull)
    spk_b = np.asarray(output_spikes, np.float32).reshape(iters, tb, o_full)

    tos = spk_b.sum(axis=1, dtype=np.float64)          # (iters, O)
    toss = tos.sum(axis=1)                             # (iters,)
    csum = np.cumsum(tos, axis=0)
    nk0 = np.asarray(N_k, np.float64).reshape(1, o_full)
    nk = np.concatenate([nk0, nk0 + csum[:-1]], axis=0)   # nk before iter i
    muw = BASE_MU / nk                                 # (iters, O)

    spks = (spk_b.astype(np.float64) * (C_CONST * muw)[:, None, :]).astype(np.float32)

    cw_terms = muw * tos
    cw_all = np.concatenate(
        [np.zeros((1, o_full)), np.cumsum(cw_terms, axis=0)], axis=0
    )  # (iters+1, O): Cw_i
    snk = nk.sum(axis=1)                               # sum_o nk_i[o]
    mub = BASE_MU_BIAS / snk                           # (iters,)
    sb1 = (mub * toss)[:, None] * tos                  # (iters, O)
    sb2 = mub * toss
    cb_all = np.concatenate([[0.0], np.cumsum(sb2)])   # (iters+1,)

    ident = np.eye(OS, dtype=np.float32)
    w_f32 = np.asarray(weights, np.float32)
    b_f32 = np.asarray(biases, np.float32)

    # psp blocks are shared by every core; +1 zero block for the
    # one-iteration-ahead corr prefetch
    psp_arr = np.zeros((nblk + 1, b, tb, ifull), np.float32)
    psp_arr[:nblk] = psp_b.reshape(nblk, b, tb, ifull)
    cb_arr = np.ascontiguousarray(
        np.broadcast_to(
            cb_all[:iters].reshape(nblk, 1, b).astype(np.float32), (nblk, OS, b)
        )
    )
    cbf_arr = np.full((OS, 1), cb_all[iters], np.float32)

    in_maps = []
    for c in range(n_cores):
        oc = slice(c * OS, (c + 1) * OS)
        in_maps.append(
            {
                "psp": psp_arr,
                "spks": np.concatenate(
                    [
                        np.ascontiguousarray(spks[:, :, oc].reshape(nblk, b, tb, OS)),
                        np.zeros((1, b, tb, OS), np.float32),
                    ],
                    axis=0,
                ),
                "w0": np.ascontiguousarray(w_f32[oc]),
                "b0": np.ascontiguousarray(b_f32[oc].reshape(OS, 1)),
                "cw": np.ascontiguousarray(
                    cw_all[:iters, oc].reshape(nblk, b, OS).transpose(0, 2, 1)
                ).astype(np.float32),
                "cb": cb_arr,
                "sb1": np.ascontiguousarray(
                    sb1[:, oc].reshape(nblk, b, OS).transpose(0, 2, 1)
                ).astype(np.float32),
                "cwf": cw_all[iters, oc].reshape(OS, 1).astype(np.float32),
                "cbf": cbf_arr,
                "ident": ident,
                "ident32": ident,
            }
        )
    return in_maps


_NC_CACHE = {}


def _get_nc(cfg: Cfg, repeats=1):
    key = (*cfg.key(), repeats)
    if key not in _NC_CACHE:
        _NC_CACHE[key] = build_nc(cfg, repeats=repeats)
    return _NC_CACHE[key]


def run_on_cores(in_maps, cfg: Cfg, trace=False, repeats=1, **kw):
    nc = _get_nc(cfg, repeats=repeats)
    return run_bass_kernel_spmd(
        nc, in_maps, core_ids=list(range(len(in_maps))), trace=trace, **kw
    )


def kernel(input_psp, output_spikes, weights, biases, N_k):
    cfg = Cfg()
    in_maps = host_prepare(input_psp, output_spikes, weights, biases, N_k, cfg)
    res = run_on_cores(in_maps, cfg)
    w = np.concatenate([res.results[c]["w_out"] for c in range(N_CORES)], axis=0)
    b = np.concatenate(
        [res.results[c]["b_out"][:, 0] for c in range(N_CORES)], axis=0
    )
    return (w, b)
